# revision 1
# baseline (speedup 1.0000x reference)
"""Trainium2 Bass kernel for pairwise radial-angular graph convolution.

Computes, for z in 0..3 (batch), a,b in 0..511 (points), i,j in 0..15:
  rel = g[z,b] - g[z,a];  d = sqrt(|rel|^2 + eps)
  rad_r = exp(-gamma*(d - c_r)^2)          (8 radial shells)
  ang   = [1, rel/d]                        (4 angular fns)
  out[z,a,i] = 1/sqrt(n) * sum_{b,r,m,j} rad_r*ang_m*W[r,m,i,j]*f[z,b,j]

v2 structure (per core: one z, one 256-wide a-half; b full 512):
  nd2[b,a] = -gamma*d^2            rank-6 factored matmul (PE, K=6, f32r)
  clamp (DVE min, hoisted); lt = ln(d^2+eps); d = exp(lt/2)  (ACT set 6)
  1/d via DVE reciprocal; rcpm = (1/d)*dmask on GPSIMD
  E7 = exp(-g*(d-c7)^2 + g*c7^2)   anchor shell (ACT Square + Exp)
  Minv = exp(-2g*dc*d), M2=Minv^2  cascade ratios (ACT exp + 1 mul)
  E_r = E_{r+1}*Minv-or-M2         bf16 2x muls, depth 4 via M2, batched
                                   in adjacent-shell pairs
  Q_r = E_r * rcpm                 bf16 muls (shells 7,6 + small-chunk 5,4
                                   on GPSIMD; rest DVE)
  Contraction REVERSED vs v1: the computed pair tensors E_r/Q_r are the
  matmul STATIONARY [128b x 128a]; the moving operand is a small bf16
  weight-column block (host f*W combos, 80 cols/shell, split hi+lo bf16
  pair for fp32-grade weights). Matmul cost scales with moving cols only.
  PSUM is pre-zeroed and all matmuls accumulate (start=False) -- regions
  interleaved in one bank with start=True corrupt accumulation here.
  out[a,i] = psum_S + psum_Vb - sum_c g_ac*psum_Vc   (DVE, broadcast
  gat = -g_a over both a-blocks at once)
  All nd2 matmuls + DVE min-clamps run up front (DVE idle window),
  then ACT/DVE/GPSIMD pipeline over b-tile chunks (1,2,1).

Sharding: 8 cores = 4 z x 2 a-halves; full output gathered on host.
"""

import math

import numpy as np

# ---------------------------------------------------------------- constants
Z, NPTS, C_IN, C_OUT = 4, 512, 16, 16
NUM_RADIAL, NUM_ANGULAR = 8, 4
MAX_R, GAMMA = 3.0, 8.0
EPS = 1e-12
N_CORES = 8
A_PER_CORE = NPTS // 2          # 256 output points per core
N_BT = NPTS // 128              # 4 b-tiles of 128
PAIRC = N_BT * A_PER_CORE       # 1024 free cols of pair tensors
CENTERS = [MAX_R * r / (NUM_RADIAL - 1) for r in range(NUM_RADIAL)]
DC = CENTERS[1] - CENTERS[0]    # shell spacing 3/7
C7SQ = GAMMA * CENTERS[7] ** 2  # exp-arg offset for the anchor shell

# knobs
CHUNK_TILES = (1, 2, 1)            # pipeline chunks over the 4 b-tiles

_CACHE = {}


def _build_program():
    import concourse.bacc as bacc
    import concourse.mybir as mybir
    import concourse.tile as tile

    f32 = mybir.dt.float32
    f32s = mybir.dt.float32r
    bf16 = mybir.dt.bfloat16
    AF = mybir.ActivationFunctionType
    ALU = mybir.AluOpType

    nc = bacc.Bacc("TRN2", target_bir_lowering=False, debug=False)

    # activation-bias constants: allocate up front, but memset inside the
    # TileContext so the dependency tracker orders them against their ACT
    # readers (avoids an all-engine barrier stalling the first DMA issue)
    const_tiles = []
    for v in (EPS, C7SQ, -CENTERS[7]):
        t = nc.alloc_sbuf_tensor(f"const-f32-{v}", [128, 1], f32)
        nc.const_aps.aps[(f32, v)] = t.ap()
        const_tiles.append((t, v))

    # ---------------- IO -------------------------------------------------
    ba_d = nc.dram_tensor("ba", [6, NPTS + A_PER_CORE], f32s,
                          kind="ExternalInput")
    gat_d = nc.dram_tensor("gat", [128, 6], f32, kind="ExternalInput")
    gh_d = nc.dram_tensor("gwh", [128, N_BT * 640], bf16, kind="ExternalInput")
    gl_d = nc.dram_tensor("gwl", [128, N_BT * 640], bf16, kind="ExternalInput")
    dmask_d = nc.dram_tensor("dmask", [128, PAIRC], bf16, kind="ExternalInput")
    out_d = nc.dram_tensor("out", [128, 32], f32, kind="ExternalOutput")

    A = A_PER_CORE

    with tile.TileContext(nc) as tc:
        with (
            tc.tile_pool(name="const", bufs=1) as cpool,
            tc.tile_pool(name="work", bufs=1) as wpool,
            tc.tile_pool(name="ndp", bufs=1, space="PSUM") as ndpool,
            tc.tile_pool(name="acc", bufs=1, space="PSUM") as accpool,
            tc.tile_pool(name="fin", bufs=1) as fpool,
        ):
            # ---------------- inputs ------------------------------------
            ba = cpool.tile([6, NPTS + A], f32s, tag="ba")
            gat = cpool.tile([128, 6], f32, tag="gat")
            gwh = cpool.tile([128, N_BT * 640], bf16, tag="gwh")
            gwl = cpool.tile([128, N_BT * 640], bf16, tag="gwl")
            dmask = cpool.tile([128, PAIRC], bf16, tag="dmask")
            nc.sync.dma_start(out=ba[:], in_=ba_d.ap())
            for t, v in const_tiles:
                nc.gpsimd.memset(t.ap(), v)
            nc.sync.dma_start(out=dmask[:], in_=dmask_d.ap())
            nc.sync.dma_start(out=gwh[:], in_=gh_d.ap())
            nc.sync.dma_start(out=gwl[:], in_=gl_d.ap())
            nc.sync.dma_start(out=gat[:], in_=gat_d.ap())
            b6 = ba[:, 0:NPTS]
            a6 = ba[:, NPTS:NPTS + A]

            # single ln+exp table set, loaded once up front
            preload = mybir.InstLoadActFuncSet(
                name=nc.get_next_instruction_name(),
                act_func_set_id=6, ins=[], outs=[])
            preload.engine = mybir.EngineType.Activation
            nc.scalar.add_instruction(preload)

            # ---------------- pair-tensor buffers -----------------------
            # E/Q shell families live in single big tiles, r-major, so
            # cascade + q multiplies batch across shells in few wide ops.
            ndps = ndpool.tile([128, PAIRC], f32, tag="ndps")
            nd2 = wpool.tile([128, PAIRC], f32, tag="nd2")
            lt = wpool.tile([128, PAIRC], f32, tag="lt")
            dd = wpool.tile([128, PAIRC], f32, tag="dd")
            sq7 = wpool.tile([128, PAIRC], f32, tag="sq7")
            rcp = wpool.tile([128, PAIRC], f32, tag="rcp")
            rcpm = wpool.tile([128, PAIRC], bf16, tag="rcpm")
            # mr: [m2 | minv] adjacent for the paired cascade step
            mr = wpool.tile([128, 2 * PAIRC], bf16, tag="mr")
            Eb = wpool.tile([128, NUM_RADIAL * PAIRC], bf16, tag="Eb")
            Qb = wpool.tile([128, NUM_RADIAL * PAIRC], bf16, tag="Qb")

            def shl(big, r, cs):
                return big[:, r * PAIRC + cs.start:r * PAIRC + cs.stop]

            def shl3(big, r0, r1, cs):        # [128, nr, cw] strided view
                return big.rearrange(
                    "p (r q) -> p r q", r=NUM_RADIAL)[:, r0:r1,
                                                      cs.start:cs.stop]

            def bcast3(t, cs, n):             # [128, cw] -> [128, n, cw]
                return t[:, cs].rearrange(
                    "p (o q) -> p o q", o=1).to_broadcast(
                        [128, n, cs.stop - cs.start])

            pcS = accpool.tile([128, 32], f32, tag="pcS")
            pcV = accpool.tile([128, 128], f32, tag="pcV")
            # pre-zeroed psum + all-accumulate matmuls (no start/stop games)
            nc.vector.memset(pcS[:], 0.0)
            nc.vector.memset(pcV[:], 0.0)

            def contraction(big, r, bt, v):
                gbase = bt * 640 + r * 80 + (16 if v else 0)
                gn = 64 if v else 16
                for blk in range(2):
                    dst = (pcV[:, blk * 64:blk * 64 + 64] if v
                           else pcS[:, blk * 16:blk * 16 + 16])
                    ssl = slice(r * PAIRC + bt * A + blk * 128,
                                r * PAIRC + bt * A + blk * 128 + 128)
                    last = (r == 0 and bt == N_BT - 1)
                    # bf16x2: hi + lo weight halves accumulate into one psum
                    nc.tensor.matmul(
                        dst, big[:, ssl], gwh[:, gbase:gbase + gn],
                        start=False, stop=False, skip_group_check=True)
                    nc.tensor.matmul(
                        dst, big[:, ssl], gwl[:, gbase:gbase + gn],
                        start=False, stop=last, skip_group_check=True)

            # all -gamma*d^2 matmuls + clamps up front: the clamps run on
            # DVE during its early idle window (waiting for ACT's d), which
            # removes one ACT op per chunk from the serial backbone
            for bt in range(N_BT):
                nc.tensor.matmul(
                    ndps[:, bt * A:(bt + 1) * A],
                    b6[:, bt * 128:(bt + 1) * 128], a6,
                    start=True, stop=True)
                bs = slice(bt * A, (bt + 1) * A)
                nc.vector.tensor_scalar_min(nd2[:, bs], ndps[:, bs], 0.0)

            t0 = 0
            for ct in CHUNK_TILES:
                tiles = range(t0, t0 + ct)
                cs = slice(t0 * A, (t0 + ct) * A)
                cw = ct * A
                t0 += ct
                nc.scalar.activation(lt[:, cs], nd2[:, cs], AF.Ln,
                                     bias=EPS, scale=-1.0 / GAMMA)
                nc.scalar.activation(dd[:, cs], lt[:, cs], AF.Exp, scale=0.5)
                csm = slice(PAIRC + cs.start, PAIRC + cs.stop)
                nc.scalar.activation(mr[:, csm], dd[:, cs], AF.Exp,
                                     scale=-2.0 * GAMMA * DC)
                # 1/d on DVE (frees the ACT backbone); rcpm on GPSIMD
                nc.vector.reciprocal(rcp[:, cs], dd[:, cs])
                nc.gpsimd.tensor_mul(rcpm[:, cs], rcp[:, cs], dmask[:, cs])
                nc.scalar.activation(sq7[:, cs], dd[:, cs], AF.Square,
                                     bias=-CENTERS[7])
                nc.scalar.activation(shl(Eb, 7, cs), sq7[:, cs], AF.Exp,
                                     bias=C7SQ, scale=-GAMMA)

                for bt in tiles:
                    contraction(Eb, 7, bt, False)
                # m2 = minv^2 into the low half of mr
                nc.vector.tensor_mul(mr[:, cs], mr[:, csm], mr[:, csm])
                # E cascade via [m2|minv] pairs: depth 4
                mrv = mr.rearrange("p (h q) -> p h q", h=2)[:, :,
                                                            cs.start:cs.stop]
                m2b = mrv[:, 0:1].to_broadcast([128, 2, cw])
                # [E5 | E6] = E7 * [m2 | minv]
                nc.vector.tensor_mul(shl3(Eb, 5, 7, cs),
                                     bcast3(Eb[:, 7 * PAIRC:], cs, 2), mrv)
                nc.gpsimd.tensor_mul(shl3(Qb, 6, 8, cs), shl3(Eb, 6, 8, cs),
                                     bcast3(rcpm, cs, 2))
                for bt in tiles:
                    contraction(Qb, 7, bt, True)
                    contraction(Eb, 6, bt, False)
                    contraction(Qb, 6, bt, True)
                nc.vector.tensor_mul(shl3(Eb, 3, 5, cs), shl3(Eb, 5, 7, cs),
                                     m2b)
                qeng = nc.gpsimd if cw <= 256 else nc.vector
                qeng.tensor_mul(shl3(Qb, 4, 6, cs), shl3(Eb, 4, 6, cs),
                                bcast3(rcpm, cs, 2))
                for bt in tiles:
                    for r in (5, 4):
                        contraction(Eb, r, bt, False)
                        contraction(Qb, r, bt, True)
                nc.vector.tensor_mul(shl3(Eb, 1, 3, cs), shl3(Eb, 3, 5, cs),
                                     m2b)
                nc.vector.tensor_mul(shl3(Qb, 2, 4, cs), shl3(Eb, 2, 4, cs),
                                     bcast3(rcpm, cs, 2))
                for bt in tiles:
                    for r in (3, 2):
                        contraction(Eb, r, bt, False)
                        contraction(Qb, r, bt, True)
                nc.vector.tensor_mul(shl(Eb, 0, cs), shl(Eb, 2, cs),
                                     mr[:, cs])
                nc.vector.tensor_mul(shl3(Qb, 0, 2, cs), shl3(Eb, 0, 2, cs),
                                     bcast3(rcpm, cs, 2))
                for bt in tiles:
                    for r in (1, 0):
                        contraction(Eb, r, bt, False)
                        contraction(Qb, r, bt, True)

            # ---------------- final combine -----------------------------
            # out = S + Vb - sum_c g_ac V_c  (gat holds -g_a); ops span both
            # a-blocks at once, <=1 PSUM operand per op
            osb = fpool.tile([128, 32], f32, tag="osb")
            w3 = fpool.tile([128, 96], f32, tag="w3")
            tmp = fpool.tile([128, 32], f32, tag="tmp")
            pcVv = pcV.rearrange("p (blk x) -> p blk x", blk=2)
            # w3 laid out c-innermost so one reduce-X sums over c
            w3v = w3.rearrange("p (blk i c) -> p blk i c", blk=2, i=16)
            nc.vector.tensor_mul(
                w3v, pcVv[:, :, 16:64].rearrange("p blk (c i) -> p blk i c",
                                                 c=3),
                gat.rearrange("p (blk o c) -> p blk o c", blk=2,
                              o=1).to_broadcast([128, 2, 16, 3]))
            nc.vector.tensor_reduce(
                tmp.rearrange("p (blk i) -> p blk i", blk=2), w3v,
                mybir.AxisListType.X, ALU.add)
            nc.vector.tensor_tensor(tmp[:], pcS[:], tmp[:], ALU.add)
            nc.vector.tensor_tensor(
                osb[:], pcVv[:, :, 0:16], tmp.rearrange(
                    "p (blk i) -> p blk i", blk=2), ALU.add)
            nc.sync.dma_start(out=out_d.ap(), in_=osb[:])

    nc.compile()
    return nc


def _host_prep(features, geometry, W, n_norm):
    """Build per-core input maps (all small host-side tensors)."""
    import ml_dtypes

    f = np.asarray(features, dtype=np.float32)
    g = np.asarray(geometry, dtype=np.float32)
    W = np.asarray(W, dtype=np.float32)
    scale = 1.0 / math.sqrt(float(n_norm))

    # fold 1/sqrt(n) and exp(-gamma c_r^2) (cascade anchor fold) into W
    Wp = W.astype(np.float64) * scale
    for r in range(NUM_RADIAL):
        Wp[r] *= math.exp(-GAMMA * CENTERS[r] ** 2)

    in_maps = []
    for core in range(N_CORES):
        z, half = core // 2, core % 2
        gz = g[z]                                    # [512, 3]
        fz = f[z]                                    # [512, 16]
        a0 = half * A_PER_CORE
        ga = gz[a0:a0 + A_PER_CORE]                  # [256, 3]

        ba = np.empty((6, NPTS + A_PER_CORE), dtype=np.float32)
        ba[0:3, :NPTS] = gz.T
        ba[3, :NPTS] = (gz * gz).sum(axis=1)
        ba[4, :NPTS] = 1.0
        ba[5, :NPTS] = 0.0
        ba[0:3, NPTS:] = 2.0 * GAMMA * ga.T
        ba[3, NPTS:] = -GAMMA
        ba[4, NPTS:] = -GAMMA * (ga * ga).sum(axis=1)
        ba[5, NPTS:] = 1.0

        # gat[p, 3*blk + c] = -g_a for a = a0 + 128*blk + p
        gat = np.empty((128, 6), dtype=np.float32)
        for blk in range(2):
            gat[:, 3 * blk:3 * blk + 3] = -ga[blk * 128:(blk + 1) * 128]

        # gw[b-part, bt*640 + r*80 + (S16 | Vb16 | V48)]
        # S_r[b,i]  = sum_j Wp[r,0,i,j] f[b,j]
        # Vb_r[b,i] = sum_cj g[b,c] Wp[r,c+1,i,j] f[b,j]
        # V_rc[b,i] = sum_j Wp[r,c+1,i,j] f[b,j]
        S = np.einsum('rij,bj->bri', Wp[:, 0], fz.astype(np.float64))
        V = np.einsum('rcij,bj->brci', Wp[:, 1:], fz.astype(np.float64))
        Vb = np.einsum('bc,brci->bri', gz.astype(np.float64), V)
        gwf = np.empty((NPTS, NUM_RADIAL, 80), dtype=np.float64)
        gwf[:, :, 0:16] = S
        gwf[:, :, 16:32] = Vb
        gwf[:, :, 32:80] = V.reshape(NPTS, NUM_RADIAL, 48)
        gwx = np.ascontiguousarray(
            gwf.reshape(N_BT, 128, NUM_RADIAL * 80)
               .transpose(1, 0, 2).reshape(128, N_BT * 640))
        gwh = gwx.astype(ml_dtypes.bfloat16)
        gwl = (gwx - gwh.astype(np.float64)).astype(ml_dtypes.bfloat16)

        dmask = np.ones((128, PAIRC), dtype=ml_dtypes.bfloat16)
        for t in range(N_BT):
            for p in range(128):
                col = t * 128 + p - a0
                if 0 <= col < A_PER_CORE:
                    dmask[p, t * A_PER_CORE + col] = 0.0

        in_maps.append({
            "ba": ba, "gat": gat, "gwh": gwh, "gwl": gwl, "dmask": dmask,
        })
    return in_maps


def kernel(features, geometry, W, n_norm):
    from concourse.bass_utils import run_bass_kernel_spmd

    if "nc" not in _CACHE:
        _CACHE["nc"] = _build_program()
    nc = _CACHE["nc"]

    in_maps = _host_prep(features, geometry, W, n_norm)
    res = run_bass_kernel_spmd(nc, in_maps, list(range(N_CORES)))

    out = np.empty((Z, NPTS, C_OUT), dtype=np.float32)
    for core in range(N_CORES):
        z, half = core // 2, core % 2
        o = res.results[core]["out"]                 # [128, 32]
        a0 = half * A_PER_CORE
        for blk in range(2):
            out[z, a0 + blk * 128:a0 + (blk + 1) * 128, :] = \
                o[:, blk * 16:(blk + 1) * 16]
    return out

